# revision 14
# baseline (speedup 1.0000x reference)
"""GCN link-prediction kernel for Trainium2, 8 NeuronCores (SPMD).

Model (from the reference):
    src,dst = edges + self-loops; dis = deg^-1/2 (deg counted over dst)
    conv(h,W,b)[n] = dis[n] * sum_{e: dst_e=n} (dis[src_e] * (h@W)[src_e]) + b
    h1 = lrelu(conv(x,W1,b1)); h2 = lrelu(conv(h1,W,b)); h3 = lrelu(conv(h2,W,b))
    z  = conv(h3,W2,b2)
    logits[k] = dot(z[a_k], z[b_k]) over concat(pos,neg) edge pairs

Device strategy:
  - Nodes sharded over 8 cores (6272 padded rows each). Each core owns the
    segment sums for its dst range (edges dst-sorted -> no cross-core adds),
    computes its slice of the next layer's table, then AllGather.
  - Per 128-edge block: dma_gather of the 128 source rows (fp16, one descriptor
    per row, thousands of rows per instruction), a one-hot selection matrix
    built with a single DVE is_equal against a resident iota tile, and one PE
    matmul accumulating into the node tile's PSUM.
  - dis[src] is folded into the gathered table, dis[dst] + bias applied once
    per node tile at PSUM evacuation (per-partition scalar ops).
  - dma_gather indices are int16, so tables are addressed via two overlapping
    32768-row windows (lo: rows [0,32768), hi: rows [Npad-32768, Npad)); the
    host sorts each tile's edges into lo/hi runs with an SPMD-shared schedule.
  - Decode phase: pairs classed by (a,b) window, z gathered from both windows,
    DVE multiply + reduce -> logits; host inverse-permutes.
"""

import math

import numpy as np

HID = 128
OUT = 64
NC = 8
LO = 32768
GMAX = 32  # max blocks per dma_gather
CT = 6     # node tiles per chunk (gather-run granularity)


# ----------------------------------------------------------------------------
# host-side preprocessing
# ----------------------------------------------------------------------------

def _wrap_idx(idx):
    """[n] int array (n % 16 == 0) -> [128, n//16] int16 SWDGE layout
    (index i at partition i%16, col i//16; replicated to all 8 Q7 groups)."""
    n = len(idx)
    w = np.empty((16, n // 16), np.int16)
    w[:, :] = np.asarray(idx, np.int64).reshape(-1, 16).T
    return np.tile(w, (8, 1))


def _pad_to(arr, n, fill):
    out = np.full(n, fill, arr.dtype)
    out[: len(arr)] = arr
    return out


def _preprocess(edge_index, pos_edge_index, neg_edge_index, N):
    """Build the SPMD-shared schedule + per-core index/metadata arrays."""
    E = edge_index.shape[1]
    src = np.concatenate([edge_index[0], np.arange(N)]).astype(np.int64)
    dst = np.concatenate([edge_index[1], np.arange(N)]).astype(np.int64)
    deg = np.bincount(dst, minlength=N).astype(np.float32)
    dis = np.where(deg > 0, 1.0 / np.sqrt(np.maximum(deg, 1e-12)), 0.0).astype(
        np.float32
    )

    NPC = N // NC               # real nodes per core
    T = math.ceil(NPC / 128)    # node tiles per core
    NPCP = T * 128              # padded nodes per core
    NPAD = NC * NPCP            # padded node space
    BASE = NPAD - LO            # hi window base
    assert NPAD <= 2 * LO, "lo/hi windows must cover the padded node space"

    def to_pad(n):  # real node id -> padded node id
        return (n // NPC) * NPCP + (n % NPC)

    src_p = to_pad(src)
    dst_core = dst // NPC
    dst_loc = dst - dst_core * NPC

    # ---- per (core, tile) edge lists, split lo/hi by padded src id
    order = np.lexsort((src_p, dst_loc // 128, dst_core))
    o_src = src_p[order]
    o_dloc = dst_loc[order]
    o_core = dst_core[order]
    o_tile = o_dloc // 128
    key = o_core * T + o_tile
    starts = np.searchsorted(key, np.arange(NC * T))
    ends = np.searchsorted(key, np.arange(NC * T), side="right")
    is_lo = o_src < LO

    cnt_lo = np.zeros((NC, T), np.int64)
    cnt_hi = np.zeros((NC, T), np.int64)
    for c in range(NC):
        for t in range(T):
            s, e = starts[c * T + t], ends[c * T + t]
            nlo = int(is_lo[s:e].sum())
            cnt_lo[c, t] = nlo
            cnt_hi[c, t] = (e - s) - nlo
    bpt_lo = [int(math.ceil(cnt_lo[:, t].max() / 128)) for t in range(T)]
    bpt_hi = [int(math.ceil(cnt_hi[:, t].max() / 128)) for t in range(T)]

    # ---- shared block stream: chunks of CT tiles, [lo run | hi run] each
    chunks = []          # list of dicts: tiles -> (t, lo_range, hi_range), groups
    nb = 0
    tile_ranges = {}     # t -> ((bs_lo, n_lo), (bs_hi, n_hi))
    groups = []          # (cls, block_start, nblocks)
    for c0 in range(0, T, CT):
        tl = list(range(c0, min(c0 + CT, T)))
        lo_start = nb
        for t in tl:
            tile_ranges[t] = [(nb, bpt_lo[t])]
            nb += bpt_lo[t]
        lo_n = nb - lo_start
        hi_start = nb
        for t in tl:
            tile_ranges[t].append((nb, bpt_hi[t]))
            nb += bpt_hi[t]
        hi_n = nb - hi_start
        gs = []
        for cls, rs, rn in ((0, lo_start, lo_n), (1, hi_start, hi_n)):
            off = 0
            while off < rn:
                g = min(GMAX, rn - off)
                gs.append((cls, rs + off, g))
                off += g
        chunks.append({"tiles": tl, "groups": gs})
        groups.extend(gs)
    NB = nb

    # block -> (group_idx, local block within group) for M-tile slicing
    blk2grp = {}
    for gi, (cls, bs, gn) in enumerate(groups):
        for j in range(gn):
            blk2grp[bs + j] = (gi, j)

    # ---- per-core gather index + dstloc arrays
    srcidx = np.zeros((NC, NB * 128), np.int64)
    dloc = np.full((NC, NB * 128), 255, np.int64)
    for c in range(NC):
        for t in range(T):
            s, e = starts[c * T + t], ends[c * T + t]
            sl = is_lo[s:e]
            src_t = o_src[s:e]
            dl_t = o_dloc[s:e] - t * 128
            for half, (bs, bn) in enumerate(tile_ranges[t]):
                if bn == 0:
                    continue
                mask = sl if half == 0 else ~sl
                sv = src_t[mask] if half == 0 else src_t[mask] - BASE
                dv = dl_t[mask]
                pad_idx = 0
                srcidx[c, bs * 128 : bs * 128 + bn * 128] = _pad_to(
                    sv, bn * 128, pad_idx
                )
                dloc[c, bs * 128 : bs * 128 + bn * 128] = _pad_to(dv, bn * 128, 255)

    srcidx_w = np.stack([_wrap_idx(srcidx[c]) for c in range(NC)])  # [NC,128,NB*8]
    dloc_t = np.stack(
        [dloc[c].reshape(NB, 128).T.astype(np.float16) for c in range(NC)]
    )  # [NC, 128, NB]

    # ---- decode schedule
    pairs = np.concatenate([pos_edge_index, neg_edge_index], axis=1).astype(np.int64)
    NP_ALL = pairs.shape[1]
    PPC = NP_ALL // NC
    a_p = to_pad(pairs[0])
    b_p = to_pad(pairs[1])
    cls_all = (a_p >= LO) * 2 + (b_p >= LO)

    dec_cnt = np.zeros((NC, 4), np.int64)
    dec_ord = []
    for c in range(NC):
        cl = cls_all[c * PPC : (c + 1) * PPC]
        ords = []
        for k in range(4):
            w = np.nonzero(cl == k)[0] + c * PPC
            dec_cnt[c, k] = len(w)
            ords.append(w)
        dec_ord.append(ords)
    nbd = [int(math.ceil(dec_cnt[:, k].max() / 128)) for k in range(4)]
    NBD = sum(nbd)

    pa = np.zeros((NC, NBD * 128), np.int64)
    pb = np.zeros((NC, NBD * 128), np.int64)
    perm = np.full((NC, NBD * 128), -1, np.int64)
    dec_ranges = []  # (cls, block_start, nblocks) runs
    bs = 0
    for k in range(4):
        dec_ranges.append((k, bs, nbd[k]))
        bs += nbd[k]
    for c in range(NC):
        for k, bs, bn in dec_ranges:
            w = dec_ord[c][k]
            av = a_p[w] - (BASE if k >= 2 else 0)
            bv = b_p[w] - (BASE if k % 2 else 0)
            pa[c, bs * 128 : bs * 128 + bn * 128] = _pad_to(av, bn * 128, 0)
            pb[c, bs * 128 : bs * 128 + bn * 128] = _pad_to(bv, bn * 128, 0)
            perm[c, bs * 128 : bs * 128 + bn * 128] = _pad_to(w, bn * 128, -1)

    dec_groups = []  # (cls, block_start, nblocks)
    for k, rs, rn in dec_ranges:
        off = 0
        while off < rn:
            g = min(GMAX, rn - off)
            dec_groups.append((k, rs + off, g))
            off += g

    pa_w = np.stack([_wrap_idx(pa[c]) for c in range(NC)])
    pb_w = np.stack([_wrap_idx(pb[c]) for c in range(NC)])

    sched = {
        "N": N, "NPC": NPC, "T": T, "NPCP": NPCP, "NPAD": NPAD, "BASE": BASE,
        "NB": NB, "NBD": NBD, "chunks": chunks, "tile_ranges": tile_ranges,
        "groups": groups, "blk2grp": blk2grp, "dec_groups": dec_groups,
    }
    percore = {
        "srcidx": srcidx_w, "dstloc": dloc_t, "paidx": pa_w, "pbidx": pb_w,
        "perm": perm,
    }
    return sched, percore, dis


# ----------------------------------------------------------------------------
# device program
# ----------------------------------------------------------------------------

def _build_program(s):
    import concourse.bacc as bacc
    import concourse.bass as bass
    import concourse.mybir as mybir
    from concourse.tile import TileContext

    F32, F16, I16 = mybir.dt.float32, mybir.dt.float16, mybir.dt.int16
    AF = mybir.ActivationFunctionType
    OP = mybir.AluOpType

    T, NPCP, NPAD, BASE = s["T"], s["NPCP"], s["NPAD"], s["BASE"]
    NB, NBD = s["NB"], s["NBD"]
    PSELW = max(
        sum(r[1] for r in s["tile_ranges"][t]) for t in range(T)
    )  # max blocks per node tile

    nc = bacc.Bacc(num_devices=NC)

    # inputs
    xT = nc.declare_dram_parameter("xT", [128, NPCP], F16, isOutput=False)
    W1f = nc.declare_dram_parameter("W1f", [128, HID], F16, isOutput=False)
    Wf = nc.declare_dram_parameter("Wf", [128, HID], F16, isOutput=False)
    W2f = nc.declare_dram_parameter("W2f", [128, OUT], F16, isOutput=False)
    # leaky-relu decomposition biases: a = 0.2*b_l, b = 0.8*b_l
    bc1a = nc.declare_dram_parameter("bc1a", [128, 1], F32, isOutput=False)
    bc1b = nc.declare_dram_parameter("bc1b", [128, 1], F32, isOutput=False)
    bcma = nc.declare_dram_parameter("bcma", [128, 1], F32, isOutput=False)
    bcmb = nc.declare_dram_parameter("bcmb", [128, 1], F32, isOutput=False)
    b2rep = nc.declare_dram_parameter("b2rep", [128, OUT], F32, isOutput=False)
    disrep = nc.declare_dram_parameter("disrep", [128, NPCP], F32, isOutput=False)
    discol = nc.declare_dram_parameter("discol", [128, T], F32, isOutput=False)
    iota = nc.declare_dram_parameter("iota", [128, 128], F16, isOutput=False)
    srcidx = nc.declare_dram_parameter("srcidx", [128, NB * 8], I16, isOutput=False)
    dstloc = nc.declare_dram_parameter("dstloc", [128, NB], F16, isOutput=False)
    paidx = nc.declare_dram_parameter("paidx", [128, NBD * 8], I16, isOutput=False)
    pbidx = nc.declare_dram_parameter("pbidx", [128, NBD * 8], I16, isOutput=False)
    lg = nc.declare_dram_parameter("lg", [128, NBD], F32, isOutput=True)

    # internal DRAM
    hs = [nc.dram_tensor(f"h{l}s", [NPCP, HID], F16) for l in (1, 2, 3)]
    hf = [
        nc.dram_tensor(f"h{l}f", [NPAD, HID], F16, addr_space="Shared")
        for l in (1, 2, 3)
    ]
    h4s = nc.dram_tensor("h4s", [NPCP, OUT], F32)
    h4f = nc.dram_tensor("h4f", [NPAD, OUT], F32, addr_space="Shared")
    zs = nc.dram_tensor("zs", [NPCP, OUT], F32)
    zf = nc.dram_tensor("zf", [NPAD, OUT], F32, addr_space="Shared")

    RG = [list(range(NC))]

    def bc3(ap, nblk, inner, mid_bcast):
        """[128, n] AP -> [128, nblk, inner] with one broadcast (step-0) dim."""
        p0 = list(ap.ap[0])
        st = ap.ap[1][0]
        if mid_bcast:  # broadcast middle dim (iota tile: [128,128] -> [128,nblk,128])
            return bass.AP(tensor=ap.tensor, offset=ap.offset,
                           ap=[p0, [0, nblk], [st, inner]])
        # broadcast inner dim (dstloc cols: [128,nblk] -> [128,nblk,128])
        return bass.AP(tensor=ap.tensor, offset=ap.offset,
                       ap=[p0, [st, nblk], [0, inner]])

    with TileContext(nc) as tc:
        with (
            tc.tile_pool(name="res", bufs=1) as res,
            tc.tile_pool(name="mg", bufs=3) as mgp,
            tc.tile_pool(name="m16", bufs=3) as m16p,
            tc.tile_pool(name="psel", bufs=4) as pselp,
            tc.tile_pool(name="ev", bufs=3) as evp,
            tc.tile_pool(name="pst", bufs=4, space="PSUM") as pstp,
            tc.tile_pool(name="ps2", bufs=2, space="PSUM") as ps2p,
        ):
            # ---- resident loads
            xT_t = res.tile([128, NPCP], F16, tag="xT")
            nc.sync.dma_start(out=xT_t[:, :], in_=xT[:, :])
            disrep_t = res.tile([128, NPCP], F32, tag="disrep")
            nc.sync.dma_start(out=disrep_t[:, :], in_=disrep[:, :])
            discol_t = res.tile([128, T], F32, tag="discol")
            nc.sync.dma_start(out=discol_t[:, :], in_=discol[:, :])
            iota_t = res.tile([128, 128], F16, tag="iota")
            nc.sync.dma_start(out=iota_t[:, :], in_=iota[:, :])
            srcidx_t = res.tile([128, NB * 8], I16, tag="srcidx")
            nc.sync.dma_start(out=srcidx_t[:, :], in_=srcidx[:, :])
            dstloc_t = res.tile([128, NB], F16, tag="dstloc")
            nc.sync.dma_start(out=dstloc_t[:, :], in_=dstloc[:, :])
            paidx_t = res.tile([128, NBD * 8], I16, tag="paidx")
            nc.sync.dma_start(out=paidx_t[:, :], in_=paidx[:, :])
            pbidx_t = res.tile([128, NBD * 8], I16, tag="pbidx")
            nc.sync.dma_start(out=pbidx_t[:, :], in_=pbidx[:, :])
            W1_t = res.tile([128, HID], F16, tag="W1")
            nc.sync.dma_start(out=W1_t[:, :], in_=W1f[:, :])
            Wm_t = res.tile([128, HID], F16, tag="Wm")
            nc.sync.dma_start(out=Wm_t[:, :], in_=Wf[:, :])
            W2_t = res.tile([128, OUT], F16, tag="W2")
            nc.sync.dma_start(out=W2_t[:, :], in_=W2f[:, :])
            bc1a_t = res.tile([128, 1], F32, tag="bc1a")
            nc.sync.dma_start(out=bc1a_t[:, :], in_=bc1a[:, :])
            bc1b_t = res.tile([128, 1], F32, tag="bc1b")
            nc.sync.dma_start(out=bc1b_t[:, :], in_=bc1b[:, :])
            bcma_t = res.tile([128, 1], F32, tag="bcma")
            nc.sync.dma_start(out=bcma_t[:, :], in_=bcma[:, :])
            bcmb_t = res.tile([128, 1], F32, tag="bcmb")
            nc.sync.dma_start(out=bcmb_t[:, :], in_=bcmb[:, :])
            b2_t = res.tile([128, OUT], F32, tag="b2")
            nc.sync.dma_start(out=b2_t[:, :], in_=b2rep[:, :])
            lg_t = res.tile([128, NBD], F32, tag="lg")

            # ---- stage 0: H1' = dis * (x @ W1), per node tile
            for t in range(T):
                ps = ps2p.tile([128, HID], F32, tag="ps2")
                nc.tensor.matmul(
                    out=ps[:, :], lhsT=xT_t[:, t * 128 : (t + 1) * 128],
                    rhs=W1_t[:, :], start=True, stop=True,
                )
                ho = evp.tile([128, HID], F16, tag="ho")
                nc.vector.tensor_scalar_mul(
                    out=ho[:, :], in0=ps[:, :], scalar1=discol_t[:, t : t + 1]
                )
                nc.sync.dma_start(
                    out=hs[0][t * 128 : (t + 1) * 128, :], in_=ho[:, :]
                )
            nc.gpsimd.collective_compute(
                "AllGather", OP.bypass, replica_groups=RG,
                ins=[hs[0][:, :].opt()], outs=[hf[0][:, :].opt()],
            )

            # ---- layers 1..4
            for l in (1, 2, 3, 4):
                fin = HID if l < 4 else OUT
                if l < 4:
                    tab = hf[l - 1]
                    tdt = F16
                else:
                    tab = h4f
                    tdt = F32
                tab_lo = tab[0:LO, :]
                tab_hi = tab[BASE : BASE + LO, :]

                for ch in s["chunks"]:
                    # gathers for the chunk
                    gtiles = {}
                    for (cls, bs, gn) in ch["groups"]:
                        mg = mgp.tile([128, GMAX * fin], tdt, tag="mg")
                        nc.gpsimd.dma_gather(
                            out_ap=mg[:, : gn * fin].rearrange(
                                "p (j f) -> p j f", f=fin
                            ),
                            in_ap=(tab_lo if cls == 0 else tab_hi),
                            idxs_ap=srcidx_t[:, bs * 8 : (bs + gn) * 8],
                            num_idxs=gn * 128,
                            num_idxs_reg=gn * 128,
                            elem_size=fin,
                            single_packet=False,
                        )
                        if l == 4:
                            m16 = m16p.tile([128, GMAX * OUT], F16, tag="m16")
                            nc.vector.tensor_copy(
                                out=m16[:, : gn * fin], in_=mg[:, : gn * fin]
                            )
                            gtiles[(bs, gn)] = m16
                        else:
                            gtiles[(bs, gn)] = mg

                    def mslice(b):
                        gi, j = s["blk2grp"][b]
                        cls, bs, gn = s["groups"][gi]
                        mt = gtiles[(bs, gn)]
                        return mt[:, j * fin : (j + 1) * fin]

                    for t in ch["tiles"]:
                        ranges = [r for r in s["tile_ranges"][t] if r[1] > 0]
                        nblk = sum(r[1] for r in ranges)
                        if nblk == 0:
                            continue
                        # one-hot selection, one is_equal per contiguous range
                        pt = pselp.tile([128, PSELW * 128], F16, tag="psel")
                        pcol = 0
                        pofs = {}
                        for (bs, bn) in ranges:
                            nc.vector.tensor_tensor(
                                out=pt[:, pcol * 128 : (pcol + bn) * 128].rearrange(
                                    "p (j f) -> p j f", f=128
                                ),
                                in0=bc3(dstloc_t[:, bs : bs + bn], bn, 128, False),
                                in1=bc3(iota_t[:, :], bn, 128, True),
                                op=OP.is_equal,
                            )
                            pofs[bs] = pcol
                            pcol += bn
                        # segment-sum matmuls
                        pst = pstp.tile([128, 128], F32, tag="pst")
                        k = 0
                        for (bs, bn) in ranges:
                            for j in range(bn):
                                psl = pt[
                                    :, (pofs[bs] + j) * 128 : (pofs[bs] + j + 1) * 128
                                ]
                                if l < 4:
                                    nc.tensor.matmul(
                                        out=pst[:, :], lhsT=mslice(bs + j), rhs=psl,
                                        start=(k == 0), stop=(k == nblk - 1),
                                    )
                                else:
                                    nc.tensor.matmul(
                                        out=pst[:, :OUT], lhsT=psl, rhs=mslice(bs + j),
                                        start=(k == 0), stop=(k == nblk - 1),
                                    )
                                k += 1
                        # evacuate
                        if l < 4:
                            xt = evp.tile([128, 128], F32, tag="xt")
                            nc.vector.tensor_tensor(
                                out=xt[:, :], in0=pst[:, :],
                                in1=disrep_t[:, t * 128 : (t + 1) * 128],
                                op=OP.mult,
                            )
                            # lrelu(x+b) = 0.2(x+b) + 0.8*relu(x+b)
                            ba = (bc1a_t if l == 1 else bcma_t)[:, :]
                            bb = (bc1b_t if l == 1 else bcmb_t)[:, :]
                            t1 = evp.tile([128, 128], F32, tag="t1")
                            nc.scalar.activation(
                                out=t1[:, :], in_=xt[:, :], func=AF.Identity,
                                bias=ba, scale=0.2,
                            )
                            t2 = evp.tile([128, 128], F32, tag="t2")
                            nc.scalar.activation(
                                out=t2[:, :], in_=xt[:, :], func=AF.Relu,
                                bias=bb, scale=0.8,
                            )
                            hp = evp.tile([128, 128], F16, tag="hp")
                            nc.vector.tensor_tensor(
                                out=hp[:, :], in0=t1[:, :], in1=t2[:, :], op=OP.add
                            )
                            fout = HID if l < 3 else OUT
                            ps2 = ps2p.tile([128, HID], F32, tag="ps2")
                            nc.tensor.matmul(
                                out=ps2[:, :fout], lhsT=hp[:, :],
                                rhs=(Wm_t[:, :] if l < 3 else W2_t[:, :]),
                                start=True, stop=True,
                            )
                            ho = evp.tile([128, HID], F16 if l < 3 else F32, tag="ho2")
                            nc.vector.tensor_scalar_mul(
                                out=ho[:, :fout], in0=ps2[:, :fout],
                                scalar1=discol_t[:, t : t + 1],
                            )
                            dstt = hs[l] if l < 3 else h4s
                            nc.sync.dma_start(
                                out=dstt[t * 128 : (t + 1) * 128, :],
                                in_=ho[:, :fout],
                            )
                        else:
                            zt = evp.tile([128, OUT], F32, tag="zt")
                            nc.vector.tensor_scalar_mul(
                                out=zt[:, :], in0=pst[:, :OUT],
                                scalar1=discol_t[:, t : t + 1],
                            )
                            z2 = evp.tile([128, OUT], F32, tag="z2")
                            nc.vector.tensor_tensor(
                                out=z2[:, :], in0=zt[:, :], in1=b2_t[:, :], op=OP.add
                            )
                            nc.sync.dma_start(
                                out=zs[t * 128 : (t + 1) * 128, :], in_=z2[:, :]
                            )

                if l < 3:
                    nc.gpsimd.collective_compute(
                        "AllGather", OP.bypass, replica_groups=RG,
                        ins=[hs[l][:, :].opt()], outs=[hf[l][:, :].opt()],
                    )
                elif l == 3:
                    nc.gpsimd.collective_compute(
                        "AllGather", OP.bypass, replica_groups=RG,
                        ins=[h4s[:, :].opt()], outs=[h4f[:, :].opt()],
                    )
                else:
                    nc.gpsimd.collective_compute(
                        "AllGather", OP.bypass, replica_groups=RG,
                        ins=[zs[:, :].opt()], outs=[zf[:, :].opt()],
                    )

            # ---- decode
            z_lo = zf[0:LO, :]
            z_hi = zf[BASE : BASE + LO, :]
            for (cls, bs, gn) in s["dec_groups"]:
                ga = mgp.tile([128, GMAX * OUT], F32, tag="mg")
                nc.gpsimd.dma_gather(
                    out_ap=ga[:, : gn * OUT].rearrange("p (j f) -> p j f", f=OUT),
                    in_ap=(z_hi if cls >= 2 else z_lo),
                    idxs_ap=paidx_t[:, bs * 8 : (bs + gn) * 8],
                    num_idxs=gn * 128, num_idxs_reg=gn * 128, elem_size=OUT,
                    single_packet=False,
                )
                gb = mgp.tile([128, GMAX * OUT], F32, tag="mgb")
                nc.gpsimd.dma_gather(
                    out_ap=gb[:, : gn * OUT].rearrange("p (j f) -> p j f", f=OUT),
                    in_ap=(z_hi if cls % 2 else z_lo),
                    idxs_ap=pbidx_t[:, bs * 8 : (bs + gn) * 8],
                    num_idxs=gn * 128, num_idxs_reg=gn * 128, elem_size=OUT,
                    single_packet=False,
                )
                pr = m16p.tile([128, GMAX * OUT], F32, tag="pr")
                nc.vector.tensor_tensor(
                    out=pr[:, : gn * OUT], in0=ga[:, : gn * OUT],
                    in1=gb[:, : gn * OUT], op=OP.mult,
                )
                nc.vector.reduce_sum(
                    out=lg_t[:, bs : bs + gn],
                    in_=pr[:, : gn * OUT].rearrange("p (j f) -> p j f", f=OUT),
                    axis=mybir.AxisListType.X,
                )
            nc.sync.dma_start(out=lg[:, :], in_=lg_t[:, :])

    return nc


# ----------------------------------------------------------------------------
# entry point
# ----------------------------------------------------------------------------

def _prepare(x, edge_index, pos_edge_index, neg_edge_index, W1, b1, W, b, W2, b2):
    N = x.shape[0]
    sched, percore, dis = _preprocess(edge_index, pos_edge_index, neg_edge_index, N)
    NPC, NPCP, T = sched["NPC"], sched["NPCP"], sched["T"]

    in_maps = []
    for c in range(NC):
        xs = np.zeros((NPCP, HID), np.float16)
        xs[:NPC] = x[c * NPC : (c + 1) * NPC].astype(np.float16)
        ds = np.zeros(NPCP, np.float32)
        ds[:NPC] = dis[c * NPC : (c + 1) * NPC]
        in_maps.append(
            {
                "xT": np.ascontiguousarray(xs.T),
                "W1f": W1.astype(np.float16),
                "Wf": W.astype(np.float16),
                "W2f": W2.astype(np.float16),
                "bc1a": 0.2 * b1.astype(np.float32).reshape(128, 1),
                "bc1b": 0.8 * b1.astype(np.float32).reshape(128, 1),
                "bcma": 0.2 * b.astype(np.float32).reshape(128, 1),
                "bcmb": 0.8 * b.astype(np.float32).reshape(128, 1),
                "b2rep": np.broadcast_to(
                    b2.astype(np.float32), (128, OUT)
                ).copy(),
                "disrep": np.broadcast_to(ds, (128, NPCP)).copy(),
                "discol": np.ascontiguousarray(ds.reshape(T, 128).T),
                "iota": np.broadcast_to(
                    np.arange(128, dtype=np.float16), (128, 128)
                ).copy(),
                "srcidx": percore["srcidx"][c],
                "dstloc": percore["dstloc"][c],
                "paidx": percore["paidx"][c],
                "pbidx": percore["pbidx"][c],
            }
        )
    return sched, percore, in_maps


def _unshard(results, sched, percore, n_pairs):
    logits = np.zeros(n_pairs, np.float32)
    for c in range(NC):
        vals = results[c]["lg"].T.reshape(-1)  # slot i=blk*128+p at [p, blk] -> transpose
        perm = percore["perm"][c]
        m = perm >= 0
        logits[perm[m]] = vals[m]
    return logits


def kernel(x, edge_index, pos_edge_index, neg_edge_index, W1, b1, W, b, W2, b2):
    from concourse.bass_utils import run_bass_kernel_spmd

    sched, percore, in_maps = _prepare(
        x, edge_index, pos_edge_index, neg_edge_index, W1, b1, W, b, W2, b2
    )
    nc = _build_program(sched)
    nc.compile()
    res = run_bass_kernel_spmd(nc, in_maps, list(range(NC)))
    n_pairs = pos_edge_index.shape[1] + neg_edge_index.shape[1]
    return _unshard(res.results, sched, percore, n_pairs)
